# revision 36
# baseline (speedup 1.0000x reference)
"""Fused FBP (ramp-filter + backprojection + flip + resize + crop) Trainium2 kernel.

The whole reference pipeline is linear in the input sinogram, so it folds into a
single constant matrix T of shape (A*DET, W*W) = (20736, 9216):

    out[n, p] = sum_k x_flat[n, k] * T[k, p]

T has a 4-fold exact symmetry (verified numerically to ~1e-5 of max):
  angle mirror:    T[(215-i, d)]    = mirror_x(T[(i, d)])        (i < 108)
  detector mirror: T[(i, 95-d)]     = rot180(T[(i, d)])          (d < 48)
so only the (i < 108, d < 48) quarter of T is streamed. Four input sets
accumulate against each streamed tile:

    A: x[i, d]       -> psum cols  0..95   fwd T
    B: x[215-i, d]   -> psum cols 96..191  fwd T
    C: x[i, 95-d]    -> psum cols  0..95   pixel-reversed T  (rot180 on outputs)
    D: x[215-i,95-d] -> psum cols 96..191  pixel-reversed T
    out = A + mirror_x(B)   (mirror_x applied on host)

Orientation: T tiles are the STATIONARY operand ([128 k, 128 pixels] -> full
128 columns enables Fast Weight Load) and x is the MOVING operand with the
A/B (and C/D) sets packed side by side (N=192).  Output lands transposed in
PSUM as [pixel, slice]; the unpermute / mirror / add epilogue runs on host.

The output-pixel axis is sharded across the 8 cores as y-mirror-closed row sets
L_c = {6c..6c+5} u {90-6c..95-6c} (so rot180 of a shard is exactly its column
reversal).  Shard pixels are additionally permuted tile-wise so that the full
1152-column reversal maps tile t -> tile 8-t with NO intra-tile reversal,
except the middle tile (t=4) whose reversed copy is appended to each chunk in
DRAM (stationary operands cannot have negative strides).  T is built on host
once (numpy) and streamed from HBM in bf16; x is replicated in bf16;
accumulation is fp32 in PSUM; output is cast to bf16 on the way out and the
final unpermute / mirror / add runs on host in fp32.
"""

import numpy as np
import ml_dtypes

N_ANGLES = 216
DET = 96
WIDTH = 96
UPSAMPLE = 1.8
PAD = 256

SLICES = 96                    # 2*1*48 sinogram slices
K = N_ANGLES * DET             # 20736 full contraction length
P_TOTAL = WIDTH * WIDTH        # 9216 output pixels per slice
NCORES = 8
PSH = P_TOTAL // NCORES        # 1152 output pixels per core
NT = PSH // 128                # 9 stationary tiles per core
A_HALF = N_ANGLES // 2         # 108
D_HALF = DET // 2              # 48
KQ = A_HALF * D_HALF           # 5184 quarter rows
KCQ = (KQ + 127) // 128        # 41 k-chunks (last one zero-padded)
KQP = KCQ * 128                # 5248 padded rows
KTAIL = 35                     # chunks 35..40 run the staggered tail
GROUPS = [1] * 4 + [3] * 10    # k-chunks per DMA group for chunks 1..34
RING = 8                       # tt ring depth
PSTRIDE = 256                  # psum cols per tile (192 used) -> bank aligned
# staggered tail: stationary-tile segments; finishing segment S completes
# psum blocks S (each stat s writes block s fwd and block 8-s rev), so the
# cast + out-DMA for a segment overlaps the next segment's matmuls.
TAIL_SEGS = [[0, 1, 2, 6, 7, 8], [3, 5], [4]]
# psum placement: position i holds block PORD[i], so mirror partners (t, 8-t)
# share a psum bank and each tail segment is bank-aligned -- the early cast
# of a finished segment then never reads a bank the PE still accumulates into
PORD = [0, 8, 1, 7, 2, 6, 3, 5, 4]
POS = {t: i for i, t in enumerate(PORD)}
SEG_POS = [(0, 6), (6, 8), (8, 9)]      # psum position ranges per segment
# o_sb column order = psum position order (host unpermutes)
OBLK = PORD

_cache = {}


def _row_set(c):
    """y rows owned by core c, ordered so rot180(shard) == reversed columns."""
    return list(range(6 * c, 6 * c + 6)) + list(range(90 - 6 * c, 96 - 6 * c))


def _tile_perm():
    """perm[128*t + j] = shard pixel held at position j of stationary tile t.

    Chosen so that global reversal s -> 1151-s maps (t, j) -> (8-t, j) for
    t != 4, and (4, j) -> (4, 127-j) for the middle tile."""
    perm = np.empty(PSH, dtype=np.int64)
    for t in range(NT):
        j = np.arange(128)
        if t <= 4:
            perm[128 * t : 128 * t + 128] = 128 * t + j
        else:
            perm[128 * t : 128 * t + 128] = PSH - 1 - (128 * (8 - t) + j)
    return perm


def _build_T_quarter():
    """T rows for angles i<108, detector d<48: (5184, 9216) float32."""
    # --- ramp filter as a circular-convolution matrix (filt = sino @ F) ---
    n = np.concatenate((np.arange(1, PAD // 2 + 1, 2), np.arange(PAD // 2 - 1, 0, -2)))
    f = np.zeros(PAD)
    f[0] = 0.25
    f[1::2] = -1.0 / (np.pi * n) ** 2
    full = 2.0 * np.real(np.fft.fft(f))
    ramp_bins = full[: PAD // 2 + 1].astype(np.float32).astype(np.float64)
    kern = np.fft.irfft(ramp_bins, n=PAD)
    s = np.pi / (2.0 * N_ANGLES)
    jj = np.arange(DET)[:, None]
    ii = np.arange(D_HALF)[None, :]
    F = (s * kern[(ii - jj) % PAD]).astype(np.float32)       # (DET j_in, 48 d_out)

    # --- backprojection weights as hat functions: W[a,d,p] = relu(1-|d-uc|)*inb ---
    angles = np.linspace(0.0, np.pi, N_ANGLES).astype(np.float32).astype(np.float64)[:A_HALF]
    grid = np.arange(WIDTH) - (WIDTH - 1) / 2.0
    ys, xs = np.meshgrid(grid, grid, indexing="ij")
    t = xs[None] * np.cos(angles)[:, None, None] + ys[None] * np.sin(angles)[:, None, None]
    u = t + (DET - 1) / 2.0                                  # (108, W, W)
    inb = ((u >= 0.0) & (u <= DET - 1)).astype(np.float32)
    uc = np.clip(u, 0.0, DET - 1).astype(np.float32)
    uc_flat = uc.reshape(A_HALF, P_TOTAL) * inb.reshape(A_HALF, P_TOTAL)
    inb_flat = inb.reshape(A_HALF, P_TOTAL)
    d = np.arange(DET, dtype=np.float32)
    T1 = np.empty((A_HALF, D_HALF, P_TOTAL), dtype=np.float32)
    for a in range(A_HALF):
        Wa = np.maximum(0.0, 1.0 - np.abs(d[:, None] - uc_flat[a][None, :])) * inb_flat[a][None, :]
        T1[a] = F.T @ Wa                                     # rows j = filtered-d 0..47

    # --- flip both spatial dims ---
    T1 = T1.reshape(A_HALF, D_HALF, WIDTH, WIDTH)[:, :, ::-1, ::-1]

    # --- upsample(1.8, linear, align_corners=False) + center-crop as one matrix ---
    up = int(WIDTH * UPSAMPLE)
    crop = (up - WIDTH) // 2
    coords = (np.arange(up) + 0.5) * (WIDTH / up) - 0.5
    coords = np.clip(coords, 0.0, WIDTH - 1)
    i0 = np.floor(coords).astype(np.int64)
    i1 = np.minimum(i0 + 1, WIDTH - 1)
    w = (coords - i0).astype(np.float32)
    C = np.zeros((WIDTH, up), dtype=np.float32)
    np.add.at(C, (i0, np.arange(up)), 1.0 - w)
    np.add.at(C, (i1, np.arange(up)), w)
    C = np.ascontiguousarray(C[:, crop : crop + WIDTH])      # (y in, Y out)

    T2 = np.tensordot(T1, C, axes=([2], [0]))                # (108, 48, X, Y)
    T2 = np.tensordot(T2, C, axes=([2], [0]))                # (108, 48, Y, X)
    return T2.reshape(KQ, P_TOTAL)


def _build_bass():
    import concourse.bass as bass
    import concourse.mybir as mybir
    from contextlib import ExitStack

    NG = len(GROUPS)               # main groups cover chunks 1..KTAIL-1
    GMAX = max(GROUPS)
    g_start = [1 + sum(GROUPS[:i]) for i in range(NG)]
    assert g_start[-1] + GROUPS[-1] == KTAIL

    nc = bass.Bass()
    xt = nc.declare_dram_parameter("xt", [128, 4 * KCQ * SLICES], mybir.dt.bfloat16, isOutput=False)
    tsh = nc.declare_dram_parameter("tsh", [KCQ, 128, PSH + 128], mybir.dt.bfloat16, isOutput=False)
    out = nc.declare_dram_parameter("out", [128, NT * 2 * SLICES], mybir.dt.bfloat16, isOutput=True)

    NTAIL = KCQ - KTAIL            # 6 staggered tail chunks
    with ExitStack() as stack:
        xt_sb = stack.enter_context(nc.sbuf_tensor([128, 4 * KCQ * SLICES], mybir.dt.bfloat16))
        tt = stack.enter_context(nc.sbuf_tensor([128, RING, GMAX, PSH + 128], mybir.dt.bfloat16))
        scratch = stack.enter_context(nc.sbuf_tensor([128, 512], mybir.dt.bfloat16))
        psum = stack.enter_context(nc.psum_tensor([128, NT * PSTRIDE], mybir.dt.float32))
        psumW = stack.enter_context(nc.psum_tensor([128, 512], mybir.dt.float32))
        o_sb = stack.enter_context(nc.sbuf_tensor([128, NT * 2 * SLICES], mybir.dt.bfloat16))
        dma_sems = [stack.enter_context(nc.semaphore(f"dma_sem{b}")) for b in range(RING)]
        xt0_sem = stack.enter_context(nc.semaphore("xt0_sem"))
        t0a_sem = stack.enter_context(nc.semaphore("t0a_sem"))
        t0b_sem = stack.enter_context(nc.semaphore("t0b_sem"))
        xt_sem = stack.enter_context(nc.semaphore("xt_sem"))
        tail_sems = [stack.enter_context(nc.semaphore(f"tail_sem{i}")) for i in range(3)]
        pe_sem = stack.enter_context(nc.semaphore("pe_sem"))
        vcopy_sem = stack.enter_context(nc.semaphore("vcopy_sem"))
        out_sem = stack.enter_context(nc.semaphore("out_sem"))
        warm_sem = stack.enter_context(nc.semaphore("warm_sem"))
        block = stack.enter_context(nc.Block())

        # xt upload: chunk 0's columns ride the sync queue first (small, so
        # the PE start gate clears early); the rest streams from the scalar
        # queue in 512-col pieces paced against PE progress so the upload
        # never steals HBM bandwidth from the T stream right when PE needs it.
        XT0 = 4 * SLICES               # 384 cols: chunk 0's A/B/C/D sets
        XP = 30
        piece = (4 * KCQ * SLICES - XT0) // XP
        assert piece == 512 and XT0 + XP * piece == 4 * KCQ * SLICES

        def xt_pieces_needed(ke):
            """pieces required before processing chunk ke (beyond xt0)"""
            return min(XP, -(-(ke * XT0) // piece))

        # sync-queue unit that first needs scalar piece i (unit 0 = chunk 0,
        # units 1..NG = main groups, unit NG+1 = tail)
        def unit_of_chunk(k):
            if k >= KTAIL:
                return NG + 1
            for g in range(NG):
                if g_start[g] <= k < g_start[g] + GROUPS[g]:
                    return g + 1
            raise AssertionError(k)

        @block.scalar
        def _(scalar):
            for i in range(XP):
                kneed = next(
                    k for k in range(1, KCQ) if xt_pieces_needed(k) > i
                )
                lead = max(0, unit_of_chunk(kneed) - 5)
                if lead > 0:
                    scalar.wait_ge(pe_sem, lead)
                scalar.dma_start(
                    out=xt_sb[:, XT0 + i * piece : XT0 + (i + 1) * piece],
                    in_=xt[:, XT0 + i * piece : XT0 + (i + 1) * piece],
                ).then_inc(xt_sem, 16)
            # out DMA pipelined per segment behind the DVE casts (on the
            # scalar queue: it is idle by then, sync still paces the ring)
            OUT_W = [2 * SLICES * len(sg) for sg in TAIL_SEGS]  # 1152, 384, 192
            off = 0
            for r, w in enumerate(OUT_W):
                scalar.wait_ge(vcopy_sem, r + 1)
                scalar.dma_start(
                    out=out[:, off : off + w], in_=o_sb[:, off : off + w]
                ).then_inc(out_sem, 16)
                off += w

        # tail chunks ride the normal ring as two more 3-chunk units
        TAIL_UNITS = [(NG + 1, KTAIL, 3), (NG + 2, KTAIL + 3, 3)]

        @block.sync
        def _(s):
            # bootstrap: xt chunk-0 cols + T chunk 0 in two halves, so the
            # first matmuls gate on the smallest possible transfers
            s.dma_start(out=xt_sb[:, 0:XT0], in_=xt[:, 0:XT0]).then_inc(
                xt0_sem, 16
            )
            s.dma_start(
                out=tt[:, 0, 0:1, 0:512],
                in_=tsh[0:1, :, 0:512].rearrange("k p n -> p k n"),
            ).then_inc(t0a_sem, 16)
            s.dma_start(
                out=tt[:, 0, 0:1, 512:1280],
                in_=tsh[0:1, :, 512:1280].rearrange("k p n -> p k n"),
            ).then_inc(t0b_sem, 16)
            for u, k0, gl in [
                (g + 1, g_start[g], GROUPS[g]) for g in range(NG)
            ] + TAIL_UNITS:
                if u >= RING:
                    s.wait_ge(pe_sem, u - RING + 1)
                s.dma_start(
                    out=tt[:, u % RING, 0:gl],
                    in_=tsh[k0 : k0 + gl].rearrange("k p n -> p k n"),
                ).then_inc(dma_sems[u % RING], 16)
            s.wait_ge(out_sem, 48)

        def pair_mms(stats, stat_ap, mv_fwd, mv_rev):
            """(psum_block, stationary, moving) pairs: stationary s feeds
            psum block s (fwd) and block 8-s (rev); one LDWEIGHTS per pair,
            hidden under the 2x192-col streams."""
            mms = []
            for s in stats:
                mms.append((s, stat_ap(s, False), mv_fwd))
                mms.append((8 - s, stat_ap(s, True), mv_rev))
            return mms

        def xt_mv(k):
            return (
                xt_sb[:, (4 * k + 0) * SLICES : (4 * k + 2) * SLICES],
                xt_sb[:, (4 * k + 2) * SLICES : (4 * k + 4) * SLICES],
            )

        def run_mms(mms, start_fn, stop_fn):
            seen = set()
            last = None
            for t, lhsT, mv in mms:
                p = POS[t] * PSTRIDE
                last = nc.tensor.matmul(
                    psum[:, p : p + 2 * SLICES],
                    lhsT,
                    mv,
                    start=start_fn(t, t not in seen),
                    stop=stop_fn(t, t in seen),
                    skip_group_check=True,
                )
                seen.add(t)
            return last

        @block.tensor
        def _(te):
            # HAM / p-state warm-up while the first transfers are in flight:
            # many short junk matmuls into a scratch PSUM bank nothing ever
            # reads, sized to keep PE continuously busy (so the DVFS ramp
            # reaches full clock) right up to when the first T chunk lands
            te.wait_ge(warm_sem, 1)
            for _ in range(20):
                nc.tensor.matmul(
                    psumW[:, 0:128],
                    scratch[:, 0:128],
                    scratch[:, 0:128],
                    start=True,
                    stop=True,
                )
            no_stop = lambda t, second: False

            def tile_stat(tile):
                def stat_ap(s, rev):
                    if rev and s == 4:
                        return tile[:, PSH : PSH + 128]
                    return tile[:, 128 * s : 128 * s + 128]

                return stat_ap

            # chunk 0: start_tensor_calc clears has_written at PSUM BANK
            # granularity (512 fp32); blocks sit 2-per-bank, so only the
            # first MM touching each bank carries start=True (a region's own
            # first write then lands via cleared has_written bits =
            # overwrite semantics).
            seen_banks = set()

            def start0(t, first_writer):
                b = min(POS[t] // 2, 4)
                if b in seen_banks:
                    return False
                seen_banks.add(b)
                return True

            te.wait_ge(t0a_sem, 16)
            te.wait_ge(xt0_sem, 16)
            mv_fwd, mv_rev = xt_mv(0)
            stat0 = tile_stat(tt[:, 0, 0])
            run_mms(pair_mms([0, 1, 2, 3], stat0, mv_fwd, mv_rev), start0, no_stop)
            te.wait_ge(t0b_sem, 16)
            last = run_mms(
                pair_mms([4, 5, 6, 7, 8], stat0, mv_fwd, mv_rev), start0, no_stop
            )
            last.then_inc(pe_sem, 1)

            no_start = lambda t, first: False
            for g in range(NG):
                u = g + 1
                te.wait_ge(dma_sems[u % RING], ((u - 1) // RING + 1) * 16)
                k0, gl = g_start[g], GROUPS[g]
                te.wait_ge(xt_sem, xt_pieces_needed(k0 + gl - 1) * 16)
                last = None
                for j in range(gl):
                    mv_fwd, mv_rev = xt_mv(k0 + j)
                    last = run_mms(
                        pair_mms(
                            list(range(NT)), tile_stat(tt[:, u % RING, j]),
                            mv_fwd, mv_rev,
                        ),
                        no_start, no_stop,
                    )
                last.then_inc(pe_sem, 1)

            # staggered tail: each segment's stationary set is mirror-closed,
            # so finishing it finalizes exactly its psum blocks
            te.wait_ge(xt_sem, XP * 16)
            for u, k0, gl in TAIL_UNITS:
                te.wait_ge(dma_sems[u % RING], ((u - 1) // RING + 1) * 16)
            for r, stats in enumerate(TAIL_SEGS):
                for kk in range(NTAIL):
                    k = KTAIL + kk
                    u, j = (NG + 1 + kk // 3, kk % 3)
                    mv_fwd, mv_rev = xt_mv(k)
                    stop_fn = (
                        (lambda t, second: second) if kk == NTAIL - 1 else no_stop
                    )
                    last = run_mms(
                        pair_mms(stats, tile_stat(tt[:, u % RING, j]), mv_fwd, mv_rev),
                        no_start, stop_fn,
                    )
                last.then_inc(pe_sem, 1)

        @block.vector
        def _(v):
            nc.vector.memset(scratch[:, :], 0).then_inc(warm_sem, 1)
            # pre-wake: a dummy op shortly before the end so the engine's
            # wake-from-idle drain happens off the critical path
            v.wait_ge(pe_sem, NG - 1)
            nc.vector.tensor_copy(scratch[:, 0:8], scratch[:, 8:16])
            psum_t = psum.rearrange("p (t c) -> p t c", c=PSTRIDE)
            o_t = o_sb.rearrange("p (t c) -> p t c", c=2 * SLICES)
            # o_sb column order is psum position order; host unpermutes
            base = NG + 1                   # pe_sem after chunk0 + all groups
            for r, (lo, hi) in enumerate(SEG_POS):
                v.wait_ge(pe_sem, base + r + 1)
                nc.vector.tensor_copy(
                    o_t[:, lo:hi],
                    psum_t[:, lo:hi, : 2 * SLICES],
                ).then_inc(vcopy_sem, 1)



    return nc


def _get_state():
    if "state" not in _cache:
        T = _build_T_quarter()
        t_bf = np.zeros((KQP, P_TOTAL), dtype=ml_dtypes.bfloat16)
        t_bf[:KQ] = T.astype(ml_dtypes.bfloat16)
        t_bf = t_bf.reshape(KCQ, 128, P_TOTAL)
        perm = _tile_perm()
        shards = []
        for c in range(NCORES):
            cols = np.array(
                [y * WIDTH + x for y in _row_set(c) for x in range(WIDTH)], dtype=np.int64
            )
            sh = t_bf[:, :, cols[perm]]
            mid_rev = sh[:, :, 512:640][:, :, ::-1]
            shards.append(np.ascontiguousarray(np.concatenate([sh, mid_rev], axis=2)))
        _cache["state"] = (shards, perm, _build_bass())
    return _cache["state"]


def _pack_lhsT(x_cols):
    """(SLICES, KQ) -> (128, KCQ, SLICES) zero-padded to KQP rows."""
    xp = np.zeros((SLICES, KQP), dtype=x_cols.dtype)
    xp[:, :KQ] = x_cols
    return xp.T.reshape(KCQ, 128, SLICES).transpose(1, 0, 2)


def _make_xt(x_flat):
    v = x_flat.reshape(SLICES, N_ANGLES, DET)
    vr = v[:, ::-1]                                     # angle 215-i at block i
    xA = v[:, :A_HALF, :D_HALF].reshape(SLICES, KQ)
    xB = vr[:, :A_HALF, :D_HALF].reshape(SLICES, KQ)
    xC = v[:, :A_HALF, ::-1][:, :, :D_HALF].reshape(SLICES, KQ)   # d -> 95-d
    xD = vr[:, :A_HALF, ::-1][:, :, :D_HALF].reshape(SLICES, KQ)
    packs = [_pack_lhsT(q) for q in (xA, xB, xC, xD)]
    return np.ascontiguousarray(
        np.stack(packs, axis=2).reshape(128, 4 * KCQ * SLICES)
    ).astype(ml_dtypes.bfloat16)


def _enable_ldw_opt():
    """Flip walrus's --enable-ldw-opt to true: consecutive matmuls sharing a
    stationary operand then skip the redundant LDWEIGHTS reload."""
    import concourse.bass_utils as bu

    if getattr(bu, "_ldwopt_patched", False):
        return
    orig = bu.run_command

    def patched(cmd, **kw):
        cmd = [
            "--enable-ldw-opt=true" if c == "--enable-ldw-opt=false" else c
            for c in cmd
        ]
        return orig(cmd, **kw)

    bu.run_command = patched
    bu._ldwopt_patched = True


def kernel(x, encoder_input_dims=None, decoder_target_shape=None, _want_perf=False):
    from concourse.bass_utils import run_bass_kernel_spmd

    _enable_ldw_opt()

    shards, perm, nc = _get_state()
    x = np.asarray(x, dtype=np.float32)
    xt_host = _make_xt(x.reshape(SLICES, K))
    in_maps = [{"xt": xt_host, "tsh": shards[c]} for c in range(NCORES)]
    res = run_bass_kernel_spmd(
        nc, in_maps, core_ids=list(range(NCORES)), trace=_want_perf
    )
    out = np.empty((SLICES, WIDTH, WIDTH), dtype=np.float32)
    inv = np.empty(PSH, dtype=np.int64)
    inv[perm] = np.arange(PSH)
    unblk = np.argsort(np.array(OBLK))   # o_sb position of psum block t
    for c in range(NCORES):
        r = np.asarray(res.results[c]["out"], dtype=np.float32).reshape(128, NT, 2 * SLICES)
        r = r[:, unblk]                  # back to block order 0..8
        # r[j, t, :192] = [A | B] for shard pixel perm[128t+j]
        ab = r.transpose(1, 0, 2).reshape(PSH, 2 * SLICES)[inv]
        sa = ab[:, :SLICES].reshape(12, WIDTH, SLICES)
        sb = ab[:, SLICES:].reshape(12, WIDTH, SLICES)
        comb = sa + sb[:, ::-1]                           # out = A + mirror_x(B)
        for t, y in enumerate(_row_set(c)):
            out[:, y, :] = comb[t].T
    out = out.reshape(2, 1, 48, WIDTH, WIDTH)
    if _want_perf:
        return out, res
    return out



# revision 38
# speedup vs baseline: 1.0939x; 1.0939x over previous
"""Fused FBP (ramp-filter + backprojection + flip + resize + crop) Trainium2 kernel.

The whole reference pipeline is linear in the input sinogram, so it folds into a
single constant matrix T of shape (A*DET, W*W) = (20736, 9216):

    out[n, p] = sum_k x_flat[n, k] * T[k, p]

T has a 4-fold exact symmetry (verified numerically to ~1e-5 of max):
  angle mirror:    T[(215-i, d)]    = mirror_x(T[(i, d)])        (i < 108)
  detector mirror: T[(i, 95-d)]     = rot180(T[(i, d)])          (d < 48)
so only the (i < 108, d < 48) quarter of T is streamed. Four input sets
accumulate against each streamed tile:

    A: x[i, d]       -> psum cols  0..95   fwd T
    B: x[215-i, d]   -> psum cols 96..191  fwd T
    C: x[i, 95-d]    -> psum cols  0..95   pixel-reversed T  (rot180 on outputs)
    D: x[215-i,95-d] -> psum cols 96..191  pixel-reversed T
    out = A + mirror_x(B)   (mirror_x applied on host)

Orientation: T tiles are the STATIONARY operand ([128 k, 128 pixels] -> full
128 columns enables Fast Weight Load) and x is the MOVING operand with the
A/B (and C/D) sets packed side by side (N=192).  Output lands transposed in
PSUM as [pixel, slice]; the unpermute / mirror / add epilogue runs on host.

The output-pixel axis is sharded across the 8 cores as y-mirror-closed row sets
L_c = {6c..6c+5} u {90-6c..95-6c} (so rot180 of a shard is exactly its column
reversal).  Shard pixels are additionally permuted tile-wise so that the full
1152-column reversal maps tile t -> tile 8-t with NO intra-tile reversal,
except the middle tile (t=4) whose reversed copy is appended to each chunk in
DRAM (stationary operands cannot have negative strides).  T is built on host
once (numpy) and streamed from HBM in bf16; x is replicated in bf16;
accumulation is fp32 in PSUM; output is cast to bf16 on the way out and the
final unpermute / mirror / add runs on host in fp32.
"""

import numpy as np
import ml_dtypes

N_ANGLES = 216
DET = 96
WIDTH = 96
UPSAMPLE = 1.8
PAD = 256

SLICES = 96                    # 2*1*48 sinogram slices
K = N_ANGLES * DET             # 20736 full contraction length
P_TOTAL = WIDTH * WIDTH        # 9216 output pixels per slice
NCORES = 8
PSH = P_TOTAL // NCORES        # 1152 output pixels per core
NT = PSH // 128                # 9 stationary tiles per core
A_HALF = N_ANGLES // 2         # 108
D_HALF = DET // 2              # 48
KQ = A_HALF * D_HALF           # 5184 quarter rows
KCQ = (KQ + 127) // 128        # 41 k-chunks (last one zero-padded)
KQP = KCQ * 128                # 5248 padded rows
KTAIL = 35                     # chunks 35..40 run the staggered tail
GROUPS = [1] * 4 + [3] * 10    # k-chunks per DMA group for chunks 1..34
RING = 8                       # tt ring depth
PSTRIDE = 256                  # psum cols per tile (192 used) -> bank aligned
# staggered tail: stationary-tile segments; finishing segment S completes
# psum blocks S (each stat s writes block s fwd and block 8-s rev), so the
# cast + out-DMA for a segment overlaps the next segment's matmuls.
TAIL_SEGS = [[0, 1, 2, 6, 7, 8], [3, 5], [4]]
# psum placement: position i holds block PORD[i], so mirror partners (t, 8-t)
# share a psum bank and each tail segment is bank-aligned -- the early cast
# of a finished segment then never reads a bank the PE still accumulates into
PORD = [0, 8, 1, 7, 2, 6, 3, 5, 4]
POS = {t: i for i, t in enumerate(PORD)}
SEG_POS = [(0, 6), (6, 8), (8, 9)]      # psum position ranges per segment
# o_sb column order = psum position order (host unpermutes)
OBLK = PORD

_cache = {}


def _row_set(c):
    """y rows owned by core c, ordered so rot180(shard) == reversed columns."""
    return list(range(6 * c, 6 * c + 6)) + list(range(90 - 6 * c, 96 - 6 * c))


def _tile_perm():
    """perm[128*t + j] = shard pixel held at position j of stationary tile t.

    Chosen so that global reversal s -> 1151-s maps (t, j) -> (8-t, j) for
    t != 4, and (4, j) -> (4, 127-j) for the middle tile."""
    perm = np.empty(PSH, dtype=np.int64)
    for t in range(NT):
        j = np.arange(128)
        if t <= 4:
            perm[128 * t : 128 * t + 128] = 128 * t + j
        else:
            perm[128 * t : 128 * t + 128] = PSH - 1 - (128 * (8 - t) + j)
    return perm


def _build_T_quarter():
    """T rows for angles i<108, detector d<48: (5184, 9216) float32."""
    # --- ramp filter as a circular-convolution matrix (filt = sino @ F) ---
    n = np.concatenate((np.arange(1, PAD // 2 + 1, 2), np.arange(PAD // 2 - 1, 0, -2)))
    f = np.zeros(PAD)
    f[0] = 0.25
    f[1::2] = -1.0 / (np.pi * n) ** 2
    full = 2.0 * np.real(np.fft.fft(f))
    ramp_bins = full[: PAD // 2 + 1].astype(np.float32).astype(np.float64)
    kern = np.fft.irfft(ramp_bins, n=PAD)
    s = np.pi / (2.0 * N_ANGLES)
    jj = np.arange(DET)[:, None]
    ii = np.arange(D_HALF)[None, :]
    F = (s * kern[(ii - jj) % PAD]).astype(np.float32)       # (DET j_in, 48 d_out)

    # --- backprojection weights as hat functions: W[a,d,p] = relu(1-|d-uc|)*inb ---
    angles = np.linspace(0.0, np.pi, N_ANGLES).astype(np.float32).astype(np.float64)[:A_HALF]
    grid = np.arange(WIDTH) - (WIDTH - 1) / 2.0
    ys, xs = np.meshgrid(grid, grid, indexing="ij")
    t = xs[None] * np.cos(angles)[:, None, None] + ys[None] * np.sin(angles)[:, None, None]
    u = t + (DET - 1) / 2.0                                  # (108, W, W)
    inb = ((u >= 0.0) & (u <= DET - 1)).astype(np.float32)
    uc = np.clip(u, 0.0, DET - 1).astype(np.float32)
    uc_flat = uc.reshape(A_HALF, P_TOTAL) * inb.reshape(A_HALF, P_TOTAL)
    inb_flat = inb.reshape(A_HALF, P_TOTAL)
    d = np.arange(DET, dtype=np.float32)
    T1 = np.empty((A_HALF, D_HALF, P_TOTAL), dtype=np.float32)
    for a in range(A_HALF):
        Wa = np.maximum(0.0, 1.0 - np.abs(d[:, None] - uc_flat[a][None, :])) * inb_flat[a][None, :]
        T1[a] = F.T @ Wa                                     # rows j = filtered-d 0..47

    # --- flip both spatial dims ---
    T1 = T1.reshape(A_HALF, D_HALF, WIDTH, WIDTH)[:, :, ::-1, ::-1]

    # --- upsample(1.8, linear, align_corners=False) + center-crop as one matrix ---
    up = int(WIDTH * UPSAMPLE)
    crop = (up - WIDTH) // 2
    coords = (np.arange(up) + 0.5) * (WIDTH / up) - 0.5
    coords = np.clip(coords, 0.0, WIDTH - 1)
    i0 = np.floor(coords).astype(np.int64)
    i1 = np.minimum(i0 + 1, WIDTH - 1)
    w = (coords - i0).astype(np.float32)
    C = np.zeros((WIDTH, up), dtype=np.float32)
    np.add.at(C, (i0, np.arange(up)), 1.0 - w)
    np.add.at(C, (i1, np.arange(up)), w)
    C = np.ascontiguousarray(C[:, crop : crop + WIDTH])      # (y in, Y out)

    T2 = np.tensordot(T1, C, axes=([2], [0]))                # (108, 48, X, Y)
    T2 = np.tensordot(T2, C, axes=([2], [0]))                # (108, 48, Y, X)
    return T2.reshape(KQ, P_TOTAL)


def _build_bass():
    import concourse.bass as bass
    import concourse.mybir as mybir
    from contextlib import ExitStack

    NG = len(GROUPS)               # main groups cover chunks 1..KTAIL-1
    GMAX = max(GROUPS)
    g_start = [1 + sum(GROUPS[:i]) for i in range(NG)]
    assert g_start[-1] + GROUPS[-1] == KTAIL

    nc = bass.Bass()
    xt = nc.declare_dram_parameter("xt", [128, 4 * KCQ * SLICES], mybir.dt.bfloat16, isOutput=False)
    tsh = nc.declare_dram_parameter("tsh", [KCQ, 128, PSH + 128], mybir.dt.bfloat16, isOutput=False)
    out = nc.declare_dram_parameter("out", [128, NT * 2 * SLICES], mybir.dt.bfloat16, isOutput=True)

    NTAIL = KCQ - KTAIL            # 6 staggered tail chunks
    with ExitStack() as stack:
        xt_sb = stack.enter_context(nc.sbuf_tensor([128, 4 * KCQ * SLICES], mybir.dt.bfloat16))
        tt = stack.enter_context(nc.sbuf_tensor([128, RING, GMAX, PSH + 128], mybir.dt.bfloat16))
        scratch = stack.enter_context(nc.sbuf_tensor([128, 512], mybir.dt.bfloat16))
        psum = stack.enter_context(nc.psum_tensor([128, NT * PSTRIDE], mybir.dt.float32))
        psumW = stack.enter_context(nc.psum_tensor([128, 512], mybir.dt.float32))
        o_sb = stack.enter_context(nc.sbuf_tensor([128, NT * 2 * SLICES], mybir.dt.bfloat16))
        dma_sems = [stack.enter_context(nc.semaphore(f"dma_sem{b}")) for b in range(RING)]
        xt0_sem = stack.enter_context(nc.semaphore("xt0_sem"))
        t0a_sem = stack.enter_context(nc.semaphore("t0a_sem"))
        t0b_sem = stack.enter_context(nc.semaphore("t0b_sem"))
        xt_sem = stack.enter_context(nc.semaphore("xt_sem"))
        tail_sems = [stack.enter_context(nc.semaphore(f"tail_sem{i}")) for i in range(3)]
        pe_sem = stack.enter_context(nc.semaphore("pe_sem"))
        vcopy_sem = stack.enter_context(nc.semaphore("vcopy_sem"))
        out_sem = stack.enter_context(nc.semaphore("out_sem"))
        warm_sem = stack.enter_context(nc.semaphore("warm_sem"))
        block = stack.enter_context(nc.Block())

        # xt upload: chunk 0's columns ride the sync queue first (small, so
        # the PE start gate clears early); the rest streams from the scalar
        # queue in 512-col pieces paced against PE progress so the upload
        # never steals HBM bandwidth from the T stream right when PE needs it.
        XT0 = 4 * SLICES               # 384 cols: chunk 0's A/B/C/D sets
        XP = 30
        piece = (4 * KCQ * SLICES - XT0) // XP
        assert piece == 512 and XT0 + XP * piece == 4 * KCQ * SLICES

        def xt_pieces_needed(ke):
            """pieces required before processing chunk ke (beyond xt0)"""
            return min(XP, -(-(ke * XT0) // piece))

        # sync-queue unit that first needs scalar piece i (unit 0 = chunk 0,
        # units 1..NG = main groups, unit NG+1 = tail)
        def unit_of_chunk(k):
            if k >= KTAIL:
                return NG + 1
            for g in range(NG):
                if g_start[g] <= k < g_start[g] + GROUPS[g]:
                    return g + 1
            raise AssertionError(k)

        @block.scalar
        def _(scalar):
            scalar.dma_start(out=xt_sb[:, 0:XT0], in_=xt[:, 0:XT0]).then_inc(
                xt0_sem, 16
            )
            for i in range(XP):
                kneed = next(
                    k for k in range(1, KCQ) if xt_pieces_needed(k) > i
                )
                lead = max(0, unit_of_chunk(kneed) - 5)
                if lead > 0:
                    scalar.wait_ge(pe_sem, lead)
                scalar.dma_start(
                    out=xt_sb[:, XT0 + i * piece : XT0 + (i + 1) * piece],
                    in_=xt[:, XT0 + i * piece : XT0 + (i + 1) * piece],
                ).then_inc(xt_sem, 16)
            # out DMA pipelined per segment behind the DVE casts (on the
            # scalar queue: it is idle by then, sync still paces the ring)
            OUT_W = [2 * SLICES * len(sg) for sg in TAIL_SEGS]  # 1152, 384, 192
            off = 0
            for r, w in enumerate(OUT_W):
                scalar.wait_ge(vcopy_sem, r + 1)
                scalar.dma_start(
                    out=out[:, off : off + w], in_=o_sb[:, off : off + w]
                ).then_inc(out_sem, 16)
                off += w

        # tail chunks ride the normal ring as two more 3-chunk units
        TAIL_UNITS = [(NG + 1, KTAIL, 3), (NG + 2, KTAIL + 3, 3)]

        @block.sync
        def _(s):
            for u, k0, gl in [(0, 0, 1)] + [
                (g + 1, g_start[g], GROUPS[g]) for g in range(NG)
            ] + TAIL_UNITS:
                if u >= RING:
                    s.wait_ge(pe_sem, u - RING + 1)
                s.dma_start(
                    out=tt[:, u % RING, 0:gl],
                    in_=tsh[k0 : k0 + gl].rearrange("k p n -> p k n"),
                ).then_inc(dma_sems[u % RING], 16)
            s.wait_ge(out_sem, 48)

        def pair_mms(stats, stat_ap, mv_fwd, mv_rev):
            """(psum_block, stationary, moving) pairs: stationary s feeds
            psum block s (fwd) and block 8-s (rev); one LDWEIGHTS per pair,
            hidden under the 2x192-col streams."""
            mms = []
            for s in stats:
                mms.append((s, stat_ap(s, False), mv_fwd))
                mms.append((8 - s, stat_ap(s, True), mv_rev))
            return mms

        def xt_mv(k):
            return (
                xt_sb[:, (4 * k + 0) * SLICES : (4 * k + 2) * SLICES],
                xt_sb[:, (4 * k + 2) * SLICES : (4 * k + 4) * SLICES],
            )

        def run_mms(mms, start_fn, stop_fn):
            seen = set()
            last = None
            for t, lhsT, mv in mms:
                p = POS[t] * PSTRIDE
                last = nc.tensor.matmul(
                    psum[:, p : p + 2 * SLICES],
                    lhsT,
                    mv,
                    start=start_fn(t, t not in seen),
                    stop=stop_fn(t, t in seen),
                    skip_group_check=True,
                )
                seen.add(t)
            return last

        @block.tensor
        def _(te):
            # HAM / p-state warm-up while the first transfers are in flight:
            # many short junk matmuls into a scratch PSUM bank nothing ever
            # reads, sized to keep PE continuously busy (so the DVFS ramp
            # reaches full clock) right up to when the first T chunk lands
            te.wait_ge(warm_sem, 1)
            for _ in range(32):
                nc.tensor.matmul(
                    psumW[:, 0:128],
                    scratch[:, 0:128],
                    scratch[:, 0:128],
                    start=True,
                    stop=True,
                )
            no_stop = lambda t, second: False

            def tile_stat(tile):
                def stat_ap(s, rev):
                    if rev and s == 4:
                        return tile[:, PSH : PSH + 128]
                    return tile[:, 128 * s : 128 * s + 128]

                return stat_ap

            # chunk 0: start_tensor_calc clears has_written at PSUM BANK
            # granularity (512 fp32); blocks sit 2-per-bank, so only the
            # first MM touching each bank carries start=True (a region's own
            # first write then lands via cleared has_written bits =
            # overwrite semantics).
            seen_banks = set()

            def start0(t, first_writer):
                b = min(POS[t] // 2, 4)
                if b in seen_banks:
                    return False
                seen_banks.add(b)
                return True

            te.wait_ge(dma_sems[0], 16)
            te.wait_ge(xt0_sem, 16)
            mv_fwd, mv_rev = xt_mv(0)
            last = run_mms(
                pair_mms(list(range(NT)), tile_stat(tt[:, 0, 0]), mv_fwd, mv_rev),
                start0, no_stop,
            )
            last.then_inc(pe_sem, 1)

            no_start = lambda t, first: False
            for g in range(NG):
                u = g + 1
                te.wait_ge(dma_sems[u % RING], (u // RING + 1) * 16)
                k0, gl = g_start[g], GROUPS[g]
                te.wait_ge(xt_sem, xt_pieces_needed(k0 + gl - 1) * 16)
                last = None
                for j in range(gl):
                    mv_fwd, mv_rev = xt_mv(k0 + j)
                    last = run_mms(
                        pair_mms(
                            list(range(NT)), tile_stat(tt[:, u % RING, j]),
                            mv_fwd, mv_rev,
                        ),
                        no_start, no_stop,
                    )
                last.then_inc(pe_sem, 1)

            # staggered tail: each segment's stationary set is mirror-closed,
            # so finishing it finalizes exactly its psum blocks
            te.wait_ge(xt_sem, XP * 16)
            for u, k0, gl in TAIL_UNITS:
                te.wait_ge(dma_sems[u % RING], (u // RING + 1) * 16)
            for r, stats in enumerate(TAIL_SEGS):
                for kk in range(NTAIL):
                    k = KTAIL + kk
                    u, j = (NG + 1 + kk // 3, kk % 3)
                    mv_fwd, mv_rev = xt_mv(k)
                    stop_fn = (
                        (lambda t, second: second) if kk == NTAIL - 1 else no_stop
                    )
                    last = run_mms(
                        pair_mms(stats, tile_stat(tt[:, u % RING, j]), mv_fwd, mv_rev),
                        no_start, stop_fn,
                    )
                last.then_inc(pe_sem, 1)

        @block.vector
        def _(v):
            nc.vector.memset(scratch[:, :], 0).then_inc(warm_sem, 1)
            # pre-wake: a dummy op shortly before the end so the engine's
            # wake-from-idle drain happens off the critical path
            v.wait_ge(pe_sem, NG - 1)
            nc.vector.tensor_copy(scratch[:, 0:8], scratch[:, 8:16])
            psum_t = psum.rearrange("p (t c) -> p t c", c=PSTRIDE)
            o_t = o_sb.rearrange("p (t c) -> p t c", c=2 * SLICES)
            # o_sb column order is psum position order; host unpermutes
            base = NG + 1                   # pe_sem after chunk0 + all groups
            for r, (lo, hi) in enumerate(SEG_POS):
                v.wait_ge(pe_sem, base + r + 1)
                nc.vector.tensor_copy(
                    o_t[:, lo:hi],
                    psum_t[:, lo:hi, : 2 * SLICES],
                ).then_inc(vcopy_sem, 1)



    return nc


def _get_state():
    if "state" not in _cache:
        T = _build_T_quarter()
        t_bf = np.zeros((KQP, P_TOTAL), dtype=ml_dtypes.bfloat16)
        t_bf[:KQ] = T.astype(ml_dtypes.bfloat16)
        t_bf = t_bf.reshape(KCQ, 128, P_TOTAL)
        perm = _tile_perm()
        shards = []
        for c in range(NCORES):
            cols = np.array(
                [y * WIDTH + x for y in _row_set(c) for x in range(WIDTH)], dtype=np.int64
            )
            sh = t_bf[:, :, cols[perm]]
            mid_rev = sh[:, :, 512:640][:, :, ::-1]
            shards.append(np.ascontiguousarray(np.concatenate([sh, mid_rev], axis=2)))
        _cache["state"] = (shards, perm, _build_bass())
    return _cache["state"]


def _pack_lhsT(x_cols):
    """(SLICES, KQ) -> (128, KCQ, SLICES) zero-padded to KQP rows."""
    xp = np.zeros((SLICES, KQP), dtype=x_cols.dtype)
    xp[:, :KQ] = x_cols
    return xp.T.reshape(KCQ, 128, SLICES).transpose(1, 0, 2)


def _make_xt(x_flat):
    v = x_flat.reshape(SLICES, N_ANGLES, DET)
    vr = v[:, ::-1]                                     # angle 215-i at block i
    xA = v[:, :A_HALF, :D_HALF].reshape(SLICES, KQ)
    xB = vr[:, :A_HALF, :D_HALF].reshape(SLICES, KQ)
    xC = v[:, :A_HALF, ::-1][:, :, :D_HALF].reshape(SLICES, KQ)   # d -> 95-d
    xD = vr[:, :A_HALF, ::-1][:, :, :D_HALF].reshape(SLICES, KQ)
    packs = [_pack_lhsT(q) for q in (xA, xB, xC, xD)]
    return np.ascontiguousarray(
        np.stack(packs, axis=2).reshape(128, 4 * KCQ * SLICES)
    ).astype(ml_dtypes.bfloat16)


def _enable_ldw_opt():
    """Flip walrus's --enable-ldw-opt to true: consecutive matmuls sharing a
    stationary operand then skip the redundant LDWEIGHTS reload."""
    import concourse.bass_utils as bu

    if getattr(bu, "_ldwopt_patched", False):
        return
    orig = bu.run_command

    def patched(cmd, **kw):
        cmd = [
            "--enable-ldw-opt=true" if c == "--enable-ldw-opt=false" else c
            for c in cmd
        ]
        return orig(cmd, **kw)

    bu.run_command = patched
    bu._ldwopt_patched = True


def kernel(x, encoder_input_dims=None, decoder_target_shape=None, _want_perf=False):
    from concourse.bass_utils import run_bass_kernel_spmd

    _enable_ldw_opt()

    shards, perm, nc = _get_state()
    x = np.asarray(x, dtype=np.float32)
    xt_host = _make_xt(x.reshape(SLICES, K))
    in_maps = [{"xt": xt_host, "tsh": shards[c]} for c in range(NCORES)]
    res = run_bass_kernel_spmd(
        nc, in_maps, core_ids=list(range(NCORES)), trace=_want_perf
    )
    out = np.empty((SLICES, WIDTH, WIDTH), dtype=np.float32)
    inv = np.empty(PSH, dtype=np.int64)
    inv[perm] = np.arange(PSH)
    unblk = np.argsort(np.array(OBLK))   # o_sb position of psum block t
    for c in range(NCORES):
        r = np.asarray(res.results[c]["out"], dtype=np.float32).reshape(128, NT, 2 * SLICES)
        r = r[:, unblk]                  # back to block order 0..8
        # r[j, t, :192] = [A | B] for shard pixel perm[128t+j]
        ab = r.transpose(1, 0, 2).reshape(PSH, 2 * SLICES)[inv]
        sa = ab[:, :SLICES].reshape(12, WIDTH, SLICES)
        sb = ab[:, SLICES:].reshape(12, WIDTH, SLICES)
        comb = sa + sb[:, ::-1]                           # out = A + mirror_x(B)
        for t, y in enumerate(_row_set(c)):
            out[:, y, :] = comb[t].T
    out = out.reshape(2, 1, 48, WIDTH, WIDTH)
    if _want_perf:
        return out, res
    return out



# revision 39
# speedup vs baseline: 1.0977x; 1.0035x over previous
"""Fused FBP (ramp-filter + backprojection + flip + resize + crop) Trainium2 kernel.

The whole reference pipeline is linear in the input sinogram, so it folds into a
single constant matrix T of shape (A*DET, W*W) = (20736, 9216):

    out[n, p] = sum_k x_flat[n, k] * T[k, p]

T has a 4-fold exact symmetry (verified numerically to ~1e-5 of max):
  angle mirror:    T[(215-i, d)]    = mirror_x(T[(i, d)])        (i < 108)
  detector mirror: T[(i, 95-d)]     = rot180(T[(i, d)])          (d < 48)
so only the (i < 108, d < 48) quarter of T is streamed. Four input sets
accumulate against each streamed tile:

    A: x[i, d]       -> psum cols  0..95   fwd T
    B: x[215-i, d]   -> psum cols 96..191  fwd T
    C: x[i, 95-d]    -> psum cols  0..95   pixel-reversed T  (rot180 on outputs)
    D: x[215-i,95-d] -> psum cols 96..191  pixel-reversed T
    out = A + mirror_x(B)   (mirror_x applied on host)

Orientation: T tiles are the STATIONARY operand ([128 k, 128 pixels] -> full
128 columns enables Fast Weight Load) and x is the MOVING operand with the
A/B (and C/D) sets packed side by side (N=192).  Output lands transposed in
PSUM as [pixel, slice]; the unpermute / mirror / add epilogue runs on host.

The output-pixel axis is sharded across the 8 cores as y-mirror-closed row sets
L_c = {6c..6c+5} u {90-6c..95-6c} (so rot180 of a shard is exactly its column
reversal).  Shard pixels are additionally permuted tile-wise so that the full
1152-column reversal maps tile t -> tile 8-t with NO intra-tile reversal,
except the middle tile (t=4) whose reversed copy is appended to each chunk in
DRAM (stationary operands cannot have negative strides).  T is built on host
once (numpy) and streamed from HBM in bf16; x is replicated in bf16;
accumulation is fp32 in PSUM; output is cast to bf16 on the way out and the
final unpermute / mirror / add runs on host in fp32.
"""

import numpy as np
import ml_dtypes

N_ANGLES = 216
DET = 96
WIDTH = 96
UPSAMPLE = 1.8
PAD = 256

SLICES = 96                    # 2*1*48 sinogram slices
K = N_ANGLES * DET             # 20736 full contraction length
P_TOTAL = WIDTH * WIDTH        # 9216 output pixels per slice
NCORES = 8
PSH = P_TOTAL // NCORES        # 1152 output pixels per core
NT = PSH // 128                # 9 stationary tiles per core
A_HALF = N_ANGLES // 2         # 108
D_HALF = DET // 2              # 48
KQ = A_HALF * D_HALF           # 5184 quarter rows
KCQ = (KQ + 127) // 128        # 41 k-chunks (last one zero-padded)
KQP = KCQ * 128                # 5248 padded rows
KTAIL = 35                     # chunks 35..40 run the staggered tail
GROUPS = [1] * 4 + [2] * 3 + [3] * 8   # k-chunks per DMA group for chunks 1..34
RING = 8                       # tt ring depth
PSTRIDE = 256                  # psum cols per tile (192 used) -> bank aligned
# staggered tail: stationary-tile segments; finishing segment S completes
# psum blocks S (each stat s writes block s fwd and block 8-s rev), so the
# cast + out-DMA for a segment overlaps the next segment's matmuls.
TAIL_SEGS = [[0, 1, 2, 6, 7, 8], [3, 5], [4]]
# psum placement: position i holds block PORD[i], so mirror partners (t, 8-t)
# share a psum bank and each tail segment is bank-aligned -- the early cast
# of a finished segment then never reads a bank the PE still accumulates into
PORD = [0, 8, 1, 7, 2, 6, 3, 5, 4]
POS = {t: i for i, t in enumerate(PORD)}
SEG_POS = [(0, 6), (6, 8), (8, 9)]      # psum position ranges per segment
# o_sb column order = psum position order (host unpermutes)
OBLK = PORD

_cache = {}


def _row_set(c):
    """y rows owned by core c, ordered so rot180(shard) == reversed columns."""
    return list(range(6 * c, 6 * c + 6)) + list(range(90 - 6 * c, 96 - 6 * c))


def _tile_perm():
    """perm[128*t + j] = shard pixel held at position j of stationary tile t.

    Chosen so that global reversal s -> 1151-s maps (t, j) -> (8-t, j) for
    t != 4, and (4, j) -> (4, 127-j) for the middle tile."""
    perm = np.empty(PSH, dtype=np.int64)
    for t in range(NT):
        j = np.arange(128)
        if t <= 4:
            perm[128 * t : 128 * t + 128] = 128 * t + j
        else:
            perm[128 * t : 128 * t + 128] = PSH - 1 - (128 * (8 - t) + j)
    return perm


def _build_T_quarter():
    """T rows for angles i<108, detector d<48: (5184, 9216) float32."""
    # --- ramp filter as a circular-convolution matrix (filt = sino @ F) ---
    n = np.concatenate((np.arange(1, PAD // 2 + 1, 2), np.arange(PAD // 2 - 1, 0, -2)))
    f = np.zeros(PAD)
    f[0] = 0.25
    f[1::2] = -1.0 / (np.pi * n) ** 2
    full = 2.0 * np.real(np.fft.fft(f))
    ramp_bins = full[: PAD // 2 + 1].astype(np.float32).astype(np.float64)
    kern = np.fft.irfft(ramp_bins, n=PAD)
    s = np.pi / (2.0 * N_ANGLES)
    jj = np.arange(DET)[:, None]
    ii = np.arange(D_HALF)[None, :]
    F = (s * kern[(ii - jj) % PAD]).astype(np.float32)       # (DET j_in, 48 d_out)

    # --- backprojection weights as hat functions: W[a,d,p] = relu(1-|d-uc|)*inb ---
    angles = np.linspace(0.0, np.pi, N_ANGLES).astype(np.float32).astype(np.float64)[:A_HALF]
    grid = np.arange(WIDTH) - (WIDTH - 1) / 2.0
    ys, xs = np.meshgrid(grid, grid, indexing="ij")
    t = xs[None] * np.cos(angles)[:, None, None] + ys[None] * np.sin(angles)[:, None, None]
    u = t + (DET - 1) / 2.0                                  # (108, W, W)
    inb = ((u >= 0.0) & (u <= DET - 1)).astype(np.float32)
    uc = np.clip(u, 0.0, DET - 1).astype(np.float32)
    uc_flat = uc.reshape(A_HALF, P_TOTAL) * inb.reshape(A_HALF, P_TOTAL)
    inb_flat = inb.reshape(A_HALF, P_TOTAL)
    d = np.arange(DET, dtype=np.float32)
    T1 = np.empty((A_HALF, D_HALF, P_TOTAL), dtype=np.float32)
    for a in range(A_HALF):
        Wa = np.maximum(0.0, 1.0 - np.abs(d[:, None] - uc_flat[a][None, :])) * inb_flat[a][None, :]
        T1[a] = F.T @ Wa                                     # rows j = filtered-d 0..47

    # --- flip both spatial dims ---
    T1 = T1.reshape(A_HALF, D_HALF, WIDTH, WIDTH)[:, :, ::-1, ::-1]

    # --- upsample(1.8, linear, align_corners=False) + center-crop as one matrix ---
    up = int(WIDTH * UPSAMPLE)
    crop = (up - WIDTH) // 2
    coords = (np.arange(up) + 0.5) * (WIDTH / up) - 0.5
    coords = np.clip(coords, 0.0, WIDTH - 1)
    i0 = np.floor(coords).astype(np.int64)
    i1 = np.minimum(i0 + 1, WIDTH - 1)
    w = (coords - i0).astype(np.float32)
    C = np.zeros((WIDTH, up), dtype=np.float32)
    np.add.at(C, (i0, np.arange(up)), 1.0 - w)
    np.add.at(C, (i1, np.arange(up)), w)
    C = np.ascontiguousarray(C[:, crop : crop + WIDTH])      # (y in, Y out)

    T2 = np.tensordot(T1, C, axes=([2], [0]))                # (108, 48, X, Y)
    T2 = np.tensordot(T2, C, axes=([2], [0]))                # (108, 48, Y, X)
    return T2.reshape(KQ, P_TOTAL)


def _build_bass():
    import concourse.bass as bass
    import concourse.mybir as mybir
    from contextlib import ExitStack

    NG = len(GROUPS)               # main groups cover chunks 1..KTAIL-1
    GMAX = max(GROUPS)
    g_start = [1 + sum(GROUPS[:i]) for i in range(NG)]
    assert g_start[-1] + GROUPS[-1] == KTAIL

    nc = bass.Bass()
    xt = nc.declare_dram_parameter("xt", [128, 4 * KCQ * SLICES], mybir.dt.bfloat16, isOutput=False)
    tsh = nc.declare_dram_parameter("tsh", [KCQ, 128, PSH + 128], mybir.dt.bfloat16, isOutput=False)
    out = nc.declare_dram_parameter("out", [128, NT * 2 * SLICES], mybir.dt.bfloat16, isOutput=True)

    NTAIL = KCQ - KTAIL            # 6 staggered tail chunks
    with ExitStack() as stack:
        xt_sb = stack.enter_context(nc.sbuf_tensor([128, 4 * KCQ * SLICES], mybir.dt.bfloat16))
        tt = stack.enter_context(nc.sbuf_tensor([128, RING, GMAX, PSH + 128], mybir.dt.bfloat16))
        scratch = stack.enter_context(nc.sbuf_tensor([128, 512], mybir.dt.bfloat16))
        psum = stack.enter_context(nc.psum_tensor([128, NT * PSTRIDE], mybir.dt.float32))
        psumW = stack.enter_context(nc.psum_tensor([128, 512], mybir.dt.float32))
        o_sb = stack.enter_context(nc.sbuf_tensor([128, NT * 2 * SLICES], mybir.dt.bfloat16))
        dma_sems = [stack.enter_context(nc.semaphore(f"dma_sem{b}")) for b in range(RING)]
        xt0_sem = stack.enter_context(nc.semaphore("xt0_sem"))
        t0a_sem = stack.enter_context(nc.semaphore("t0a_sem"))
        t0b_sem = stack.enter_context(nc.semaphore("t0b_sem"))
        xt_sem = stack.enter_context(nc.semaphore("xt_sem"))
        tail_sems = [stack.enter_context(nc.semaphore(f"tail_sem{i}")) for i in range(3)]
        pe_sem = stack.enter_context(nc.semaphore("pe_sem"))
        vcopy_sem = stack.enter_context(nc.semaphore("vcopy_sem"))
        out_sem = stack.enter_context(nc.semaphore("out_sem"))
        warm_sem = stack.enter_context(nc.semaphore("warm_sem"))
        block = stack.enter_context(nc.Block())

        # xt upload: chunk 0's columns ride the sync queue first (small, so
        # the PE start gate clears early); the rest streams from the scalar
        # queue in 512-col pieces paced against PE progress so the upload
        # never steals HBM bandwidth from the T stream right when PE needs it.
        XT0 = 4 * SLICES               # 384 cols: chunk 0's A/B/C/D sets
        XP = 30
        piece = (4 * KCQ * SLICES - XT0) // XP
        assert piece == 512 and XT0 + XP * piece == 4 * KCQ * SLICES

        def xt_pieces_needed(ke):
            """pieces required before processing chunk ke (beyond xt0)"""
            return min(XP, -(-(ke * XT0) // piece))

        # sync-queue unit that first needs scalar piece i (unit 0 = chunk 0,
        # units 1..NG = main groups, unit NG+1 = tail)
        def unit_of_chunk(k):
            if k >= KTAIL:
                return NG + 1
            for g in range(NG):
                if g_start[g] <= k < g_start[g] + GROUPS[g]:
                    return g + 1
            raise AssertionError(k)

        @block.scalar
        def _(scalar):
            scalar.dma_start(out=xt_sb[:, 0:XT0], in_=xt[:, 0:XT0]).then_inc(
                xt0_sem, 16
            )
            for i in range(XP):
                kneed = next(
                    k for k in range(1, KCQ) if xt_pieces_needed(k) > i
                )
                lead = max(0, unit_of_chunk(kneed) - 5)
                if lead > 0:
                    scalar.wait_ge(pe_sem, lead)
                scalar.dma_start(
                    out=xt_sb[:, XT0 + i * piece : XT0 + (i + 1) * piece],
                    in_=xt[:, XT0 + i * piece : XT0 + (i + 1) * piece],
                ).then_inc(xt_sem, 16)
            # out DMA pipelined per segment behind the DVE casts (on the
            # scalar queue: it is idle by then, sync still paces the ring)
            OUT_W = [2 * SLICES * len(sg) for sg in TAIL_SEGS]  # 1152, 384, 192
            off = 0
            for r, w in enumerate(OUT_W):
                scalar.wait_ge(vcopy_sem, r + 1)
                scalar.dma_start(
                    out=out[:, off : off + w], in_=o_sb[:, off : off + w]
                ).then_inc(out_sem, 16)
                off += w

        # tail chunks ride the normal ring as two more 3-chunk units
        TAIL_UNITS = [(NG + 1, KTAIL, 3), (NG + 2, KTAIL + 3, 3)]

        @block.sync
        def _(s):
            for u, k0, gl in [(0, 0, 1)] + [
                (g + 1, g_start[g], GROUPS[g]) for g in range(NG)
            ] + TAIL_UNITS:
                if u >= RING:
                    s.wait_ge(pe_sem, u - RING + 1)
                s.dma_start(
                    out=tt[:, u % RING, 0:gl],
                    in_=tsh[k0 : k0 + gl].rearrange("k p n -> p k n"),
                ).then_inc(dma_sems[u % RING], 16)
            s.wait_ge(out_sem, 48)

        def pair_mms(stats, stat_ap, mv_fwd, mv_rev):
            """(psum_block, stationary, moving) pairs: stationary s feeds
            psum block s (fwd) and block 8-s (rev); one LDWEIGHTS per pair,
            hidden under the 2x192-col streams."""
            mms = []
            for s in stats:
                mms.append((s, stat_ap(s, False), mv_fwd))
                mms.append((8 - s, stat_ap(s, True), mv_rev))
            return mms

        def xt_mv(k):
            return (
                xt_sb[:, (4 * k + 0) * SLICES : (4 * k + 2) * SLICES],
                xt_sb[:, (4 * k + 2) * SLICES : (4 * k + 4) * SLICES],
            )

        def run_mms(mms, start_fn, stop_fn):
            seen = set()
            last = None
            for t, lhsT, mv in mms:
                p = POS[t] * PSTRIDE
                last = nc.tensor.matmul(
                    psum[:, p : p + 2 * SLICES],
                    lhsT,
                    mv,
                    start=start_fn(t, t not in seen),
                    stop=stop_fn(t, t in seen),
                    skip_group_check=True,
                )
                seen.add(t)
            return last

        @block.tensor
        def _(te):
            # HAM / p-state warm-up while the first transfers are in flight:
            # many short junk matmuls into a scratch PSUM bank nothing ever
            # reads, sized to keep PE continuously busy (so the DVFS ramp
            # reaches full clock) right up to when the first T chunk lands
            te.wait_ge(warm_sem, 1)
            for _ in range(32):
                nc.tensor.matmul(
                    psumW[:, 0:128],
                    scratch[:, 0:128],
                    scratch[:, 0:128],
                    start=True,
                    stop=True,
                )
            no_stop = lambda t, second: False

            def tile_stat(tile):
                def stat_ap(s, rev):
                    if rev and s == 4:
                        return tile[:, PSH : PSH + 128]
                    return tile[:, 128 * s : 128 * s + 128]

                return stat_ap

            # chunk 0: start_tensor_calc clears has_written at PSUM BANK
            # granularity (512 fp32); blocks sit 2-per-bank, so only the
            # first MM touching each bank carries start=True (a region's own
            # first write then lands via cleared has_written bits =
            # overwrite semantics).
            seen_banks = set()

            def start0(t, first_writer):
                b = min(POS[t] // 2, 4)
                if b in seen_banks:
                    return False
                seen_banks.add(b)
                return True

            te.wait_ge(dma_sems[0], 16)
            te.wait_ge(xt0_sem, 16)
            mv_fwd, mv_rev = xt_mv(0)
            last = run_mms(
                pair_mms(list(range(NT)), tile_stat(tt[:, 0, 0]), mv_fwd, mv_rev),
                start0, no_stop,
            )
            last.then_inc(pe_sem, 1)

            no_start = lambda t, first: False
            for g in range(NG):
                u = g + 1
                te.wait_ge(dma_sems[u % RING], (u // RING + 1) * 16)
                k0, gl = g_start[g], GROUPS[g]
                te.wait_ge(xt_sem, xt_pieces_needed(k0 + gl - 1) * 16)
                last = None
                for j in range(gl):
                    mv_fwd, mv_rev = xt_mv(k0 + j)
                    last = run_mms(
                        pair_mms(
                            list(range(NT)), tile_stat(tt[:, u % RING, j]),
                            mv_fwd, mv_rev,
                        ),
                        no_start, no_stop,
                    )
                last.then_inc(pe_sem, 1)

            # staggered tail: each segment's stationary set is mirror-closed,
            # so finishing it finalizes exactly its psum blocks
            te.wait_ge(xt_sem, XP * 16)
            for u, k0, gl in TAIL_UNITS:
                te.wait_ge(dma_sems[u % RING], (u // RING + 1) * 16)
            for r, stats in enumerate(TAIL_SEGS):
                for kk in range(NTAIL):
                    k = KTAIL + kk
                    u, j = (NG + 1 + kk // 3, kk % 3)
                    mv_fwd, mv_rev = xt_mv(k)
                    stop_fn = (
                        (lambda t, second: second) if kk == NTAIL - 1 else no_stop
                    )
                    last = run_mms(
                        pair_mms(stats, tile_stat(tt[:, u % RING, j]), mv_fwd, mv_rev),
                        no_start, stop_fn,
                    )
                last.then_inc(pe_sem, 1)

        @block.vector
        def _(v):
            nc.vector.memset(scratch[:, :], 0).then_inc(warm_sem, 1)
            # pre-wake: a dummy op shortly before the end so the engine's
            # wake-from-idle drain happens off the critical path
            v.wait_ge(pe_sem, NG - 1)
            nc.vector.tensor_copy(scratch[:, 0:8], scratch[:, 8:16])
            psum_t = psum.rearrange("p (t c) -> p t c", c=PSTRIDE)
            o_t = o_sb.rearrange("p (t c) -> p t c", c=2 * SLICES)
            # o_sb column order is psum position order; host unpermutes
            base = NG + 1                   # pe_sem after chunk0 + all groups
            for r, (lo, hi) in enumerate(SEG_POS):
                v.wait_ge(pe_sem, base + r + 1)
                nc.vector.tensor_copy(
                    o_t[:, lo:hi],
                    psum_t[:, lo:hi, : 2 * SLICES],
                ).then_inc(vcopy_sem, 1)



    return nc


def _get_state():
    if "state" not in _cache:
        T = _build_T_quarter()
        t_bf = np.zeros((KQP, P_TOTAL), dtype=ml_dtypes.bfloat16)
        t_bf[:KQ] = T.astype(ml_dtypes.bfloat16)
        t_bf = t_bf.reshape(KCQ, 128, P_TOTAL)
        perm = _tile_perm()
        shards = []
        for c in range(NCORES):
            cols = np.array(
                [y * WIDTH + x for y in _row_set(c) for x in range(WIDTH)], dtype=np.int64
            )
            sh = t_bf[:, :, cols[perm]]
            mid_rev = sh[:, :, 512:640][:, :, ::-1]
            shards.append(np.ascontiguousarray(np.concatenate([sh, mid_rev], axis=2)))
        _cache["state"] = (shards, perm, _build_bass())
    return _cache["state"]


def _pack_lhsT(x_cols):
    """(SLICES, KQ) -> (128, KCQ, SLICES) zero-padded to KQP rows."""
    xp = np.zeros((SLICES, KQP), dtype=x_cols.dtype)
    xp[:, :KQ] = x_cols
    return xp.T.reshape(KCQ, 128, SLICES).transpose(1, 0, 2)


def _make_xt(x_flat):
    v = x_flat.reshape(SLICES, N_ANGLES, DET)
    vr = v[:, ::-1]                                     # angle 215-i at block i
    xA = v[:, :A_HALF, :D_HALF].reshape(SLICES, KQ)
    xB = vr[:, :A_HALF, :D_HALF].reshape(SLICES, KQ)
    xC = v[:, :A_HALF, ::-1][:, :, :D_HALF].reshape(SLICES, KQ)   # d -> 95-d
    xD = vr[:, :A_HALF, ::-1][:, :, :D_HALF].reshape(SLICES, KQ)
    packs = [_pack_lhsT(q) for q in (xA, xB, xC, xD)]
    return np.ascontiguousarray(
        np.stack(packs, axis=2).reshape(128, 4 * KCQ * SLICES)
    ).astype(ml_dtypes.bfloat16)


def _enable_ldw_opt():
    """Flip walrus's --enable-ldw-opt to true: consecutive matmuls sharing a
    stationary operand then skip the redundant LDWEIGHTS reload."""
    import concourse.bass_utils as bu

    if getattr(bu, "_ldwopt_patched", False):
        return
    orig = bu.run_command

    def patched(cmd, **kw):
        cmd = [
            "--enable-ldw-opt=true" if c == "--enable-ldw-opt=false" else c
            for c in cmd
        ]
        return orig(cmd, **kw)

    bu.run_command = patched
    bu._ldwopt_patched = True


def kernel(x, encoder_input_dims=None, decoder_target_shape=None, _want_perf=False):
    from concourse.bass_utils import run_bass_kernel_spmd

    _enable_ldw_opt()

    shards, perm, nc = _get_state()
    x = np.asarray(x, dtype=np.float32)
    xt_host = _make_xt(x.reshape(SLICES, K))
    in_maps = [{"xt": xt_host, "tsh": shards[c]} for c in range(NCORES)]
    res = run_bass_kernel_spmd(
        nc, in_maps, core_ids=list(range(NCORES)), trace=_want_perf
    )
    out = np.empty((SLICES, WIDTH, WIDTH), dtype=np.float32)
    inv = np.empty(PSH, dtype=np.int64)
    inv[perm] = np.arange(PSH)
    unblk = np.argsort(np.array(OBLK))   # o_sb position of psum block t
    for c in range(NCORES):
        r = np.asarray(res.results[c]["out"], dtype=np.float32).reshape(128, NT, 2 * SLICES)
        r = r[:, unblk]                  # back to block order 0..8
        # r[j, t, :192] = [A | B] for shard pixel perm[128t+j]
        ab = r.transpose(1, 0, 2).reshape(PSH, 2 * SLICES)[inv]
        sa = ab[:, :SLICES].reshape(12, WIDTH, SLICES)
        sb = ab[:, SLICES:].reshape(12, WIDTH, SLICES)
        comb = sa + sb[:, ::-1]                           # out = A + mirror_x(B)
        for t, y in enumerate(_row_set(c)):
            out[:, y, :] = comb[t].T
    out = out.reshape(2, 1, 48, WIDTH, WIDTH)
    if _want_perf:
        return out, res
    return out

